# revision 1
# baseline (speedup 1.0000x reference)
"""AWQ int4 dequant linear + LoRA, tensor-parallel over 8 TRN2 NeuronCores.

Math (per reference):
  W[i,o] = (w4[i,o] - z4[g(i),o]) * s[g(i),o],  g(i) = i // 128
  out = x @ W + 2.0 * (x @ lora_A.T) @ lora_B.T

Sharding: column-parallel — each core owns 1376 of the 11008 output features
(qweight/qzeros/scales/lora_B sharded on the out dim; x, lora_A replicated).

Device algorithm (per core):
  - qweight nibbles pack along OUT: byte b of a row holds outputs (2b, 2b+1).
    Unpack on DVE at int32 granularity: lo32 = q & 0x0F0F0F0F (even outputs),
    hi32 = (q >> 4) & 0x0F0F0F0F (odd outputs); int8-view -> fp16 on ACT.
  - Scales fold into W as W' = nib * s. The -z*s term is folded out exactly:
      x @ W = x @ (nib * s) - xsum_g @ (z4 * s),  xsum_g[t] = sum_{i in g} x[t,i]
    The correction is a tiny K=32 matmul.
  - Row permutation trick: contraction chunk c takes rows
    i = 128*(p//4) + 4c + (p%4), so every chunk sees the same group layout
    (group = p//4) and ONE host-replicated scale tile [128, 688] serves all
    32 chunks (no on-device scale broadcast needed).
  - xsum and lora1 = x @ lora_A.T come from one aux matmul against [E | A.T].
  - Even/odd output columns are interleaved during the PSUM->SBUF drain.

All heavy tensors are fp16 (exact for nibbles; x/scale rounding ~5e-4 rel).
"""

import sys
import numpy as np

if "/opt/trn_rl_repo" not in sys.path:
    sys.path.insert(0, "/opt/trn_rl_repo")

import concourse.bass as bass
import concourse.mybir as mybir
import concourse.tile as tile
from concourse import bacc
from concourse.bass_utils import run_bass_kernel_spmd

TOKENS, IN_F, OUT_F = 256, 4096, 11008
GROUP = 128
NG = IN_F // GROUP            # 32 groups
NCORES = 8
OPC = OUT_F // NCORES         # 1376 outputs per core
WPC = OPC // 8                # 172 int32 words per core
BPC = OPC // 2                # 688 bytes per row per core (=#even outputs)
NCHUNK = 32                   # contraction chunks of 128 rows
CB = 4                        # chunks per DVE batch
NB = NCHUNK // CB             # 8 batches
AUXW = NG + 16                # 48 aux columns: [E(32) | lora_A.T(16)]

_cache = {}


def _row_perm():
    """perm[c, p] -> original row i = 128*(p//4) + 4c + p%4."""
    p = np.arange(128)
    c = np.arange(NCHUNK)
    return (128 * (p[None, :] // 4) + 4 * c[:, None] + (p[None, :] % 4))


def build_program(compile_=True, reps=1):
    fp16 = mybir.dt.float16
    f32 = mybir.dt.float32
    i32 = mybir.dt.int32
    i8 = mybir.dt.int8
    Alu = mybir.AluOpType

    # Bacc (not plain Bass): its compile() runs generate_event_semaphores,
    # which splits multi-wait instructions into the 1-wait-per-instruction
    # form the TRN2 ISA requires.
    nc = bacc.Bacc("TRN2", target_bir_lowering=False)

    xT_d = nc.dram_tensor("xt", [128, NCHUNK * TOKENS], fp16, kind="ExternalInput")
    qw_d = nc.dram_tensor("qw", [128, NCHUNK * WPC], i32, kind="ExternalInput")
    srep_d = nc.dram_tensor("srep", [128, 2 * BPC], fp16, kind="ExternalInput")
    ae_d = nc.dram_tensor("ae", [128, NCHUNK * AUXW], fp16, kind="ExternalInput")
    szn_d = nc.dram_tensor("szn", [NG, OPC], fp16, kind="ExternalInput")
    bt2_d = nc.dram_tensor("bt2", [16, OPC], fp16, kind="ExternalInput")
    out_d = nc.dram_tensor("out", [TOKENS, OPC], f32, kind="ExternalOutput")

    with tile.TileContext(nc) as tc:
        with tc.tile_pool(name="res", bufs=1) as res, \
             tc.tile_pool(name="work", bufs=2) as work, \
             tc.tile_pool(name="ps", bufs=1, space="PSUM") as ps:

            for _rep in range(reps):
                # ---- resident loads (big tensors split per batch so batch-0
                # compute starts after ~1.5 MB instead of ~7 MB of DMA) ----
                # Small resident tensors go on the ACT HWDGE ring
                # (nc.scalar) so they don't serialize ahead of the xT/qw
                # stream on the SP ring — the two rings run in parallel.
                ae = res.tile([128, NCHUNK * AUXW], fp16)
                nc.scalar.dma_start(ae[:], ae_d[:, :])
                srep = res.tile([128, 2 * BPC], fp16)
                nc.scalar.dma_start(srep[:], srep_d[:, :])
                szn = res.tile([NG, OPC], fp16)
                nc.scalar.dma_start(szn[:], szn_d[:, :])
                bt2 = res.tile([16, OPC], fp16)
                nc.scalar.dma_start(bt2[:], bt2_d[:, :])
                xT = res.tile([128, NCHUNK * TOKENS], fp16)
                qw = res.tile([128, NCHUNK * WPC], i32)
                for b in range(NB):
                    xs_ = slice(b * CB * TOKENS, (b + 1) * CB * TOKENS)
                    qs = slice(b * CB * WPC, (b + 1) * CB * WPC)
                    nc.sync.dma_start(xT[:, xs_], xT_d[:, xs_])
                    nc.sync.dma_start(qw[:, qs], qw_d[:, qs])

                # ---- psum accumulators (bank = 512 f32) ----
                pev = [ps.tile([128, 512], f32, name=f"pev{m}") for m in range(2)]
                pod = [ps.tile([128, 512], f32, name=f"pod{m}") for m in range(2)]
                ptl = [ps.tile([128, 352], f32, name=f"ptl{m}") for m in range(2)]
                # Aux accumulator: rows 0:32 = xsum (E cols), 32:48 = lora1
                # (A.T cols). Its lora rows get DMA-shifted to a base-0 tile
                # before use: matmul accumulation chains with mixed operand
                # base partitions fault the PE on this silicon.
                paux = ps.tile([AUXW, TOKENS], f32)

                def sbc(lo, hi):
                    return srep[:, lo:hi].unsqueeze(1).to_broadcast((128, CB, hi - lo))

                # ---- phase 1: aux matmuls (xsum via E, lora1 via A.T) need
                # only xT/ae — they fill the PE while the dequant pipeline
                # (DMA -> unpack -> convert -> scale) produces batch 0's W.
                for c in range(NCHUNK):
                    st = (c == 0)
                    sp = (c == NCHUNK - 1)
                    nc.tensor.matmul(
                        paux[:], ae[:, c * AUXW:(c + 1) * AUXW],
                        xT[:, c * TOKENS:(c + 1) * TOKENS], start=st, stop=sp)

                aux_sb = res.tile([AUXW, TOKENS], fp16)
                lo_sb = res.tile([16, TOKENS], fp16)

                # ---- phase 3: dequant + base matmuls ----
                for b in range(NB):
                    wslice = qw[:, b * CB * WPC:(b + 1) * CB * WPC]  # [128, 688] i32
                    # int8-typed tiles written through an int32 view keep the
                    # access patterns 2-D (a bitcast int32->int8 read would be 3-D).
                    lo8 = work.tile([128, CB * BPC], i8, tag="lo8")
                    hi8 = work.tile([128, CB * BPC], i8, tag="hi8")
                    nc.vector.tensor_scalar(
                        lo8[:].bitcast(i32), wslice, 0x0F0F0F0F, None,
                        Alu.bitwise_and)
                    nc.vector.tensor_scalar(
                        hi8[:].bitcast(i32), wslice, 4, 0x0F0F0F0F,
                        Alu.logical_shift_right, Alu.bitwise_and)

                    cv_ev = work.tile([128, CB * BPC], fp16, tag="cv_ev", bufs=3)
                    cv_od = work.tile([128, CB * BPC], fp16, tag="cv_od", bufs=3)
                    nc.scalar.copy(cv_ev[:], lo8[:])
                    # every other odd-half convert goes to DVE to balance
                    # ACT/DVE; batch 0 splits across both engines so the first
                    # W tile materializes as early as possible.
                    if b % 2 == 1:
                        nc.scalar.copy(cv_od[:], hi8[:])
                    else:
                        nc.vector.tensor_copy(cv_od[:], hi8[:])

                    # W layout per chunk: [ev 0:512 | od 512:1024 | evtail | odtail]
                    # so each (chunk, m) is exactly 3 matmuls into 3 psum banks.
                    wall = work.tile([128, CB * OPC], fp16, tag="wall", bufs=3)
                    wv = wall[:].rearrange("p (c o) -> p c o", c=CB)
                    cev = cv_ev[:].rearrange("p (c o) -> p c o", c=CB)
                    cod = cv_od[:].rearrange("p (c o) -> p c o", c=CB)
                    nc.vector.tensor_tensor(
                        wv[:, :, 0:512], cev[:, :, 0:512], sbc(0, 512), Alu.mult)
                    nc.vector.tensor_tensor(
                        wv[:, :, 512:1024], cod[:, :, 0:512],
                        sbc(BPC, BPC + 512), Alu.mult)
                    nc.vector.tensor_tensor(
                        wv[:, :, 1024:1200], cev[:, :, 512:BPC],
                        sbc(512, BPC), Alu.mult)
                    nc.vector.tensor_tensor(
                        wv[:, :, 1200:1376], cod[:, :, 512:BPC],
                        sbc(BPC + 512, 2 * BPC), Alu.mult)

                    for j in range(CB):
                        c = b * CB + j
                        st = (c == 0)
                        sp = (c == NCHUNK - 1)
                        w0 = j * OPC
                        for m in range(2):
                            lhsT = xT[:, c * TOKENS + m * 128: c * TOKENS + (m + 1) * 128]
                            nc.tensor.matmul(
                                pev[m][:], lhsT, wall[:, w0:w0 + 512],
                                start=st, stop=sp)
                            nc.tensor.matmul(
                                pod[m][:], lhsT, wall[:, w0 + 512:w0 + 1024],
                                start=st, stop=sp)
                            nc.tensor.matmul(
                                ptl[m][:], lhsT, wall[:, w0 + 1024:w0 + 1376],
                                start=st, stop=sp)
                    if b == 0:
                        # correction operands + xsum corrections ride here
                        # (psum accumulation commutes): after batch 0 no
                        # engine's phase-3 pipeline is blocked waiting for the
                        # aux accumulator to close.
                        nc.scalar.copy(aux_sb[:], paux[:])
                        # ACT ring: the SP ring still has queued xT/qw input
                        # transfers ahead of this tiny partition-shift.
                        nc.scalar.dma_start(lo_sb[:], aux_sb[NG:AUXW, :])
                        for m in range(2):
                            xs = aux_sb[0:NG, m * 128:(m + 1) * 128]
                            nc.tensor.matmul(pev[m][:], xs, szn[:, 0:512],
                                             start=False, stop=False)
                            nc.tensor.matmul(pod[m][:], xs, szn[:, 512:1024],
                                             start=False, stop=False)
                            nc.tensor.matmul(ptl[m][:], xs, szn[:, 1024:1376],
                                             start=False, stop=False)
                    if b == 1:
                        # lora corrections one batch later: the lo_sb
                        # partition-shift DMA has landed by now.
                        for m in range(2):
                            lo = lo_sb[:][:, m * 128:(m + 1) * 128]
                            nc.tensor.matmul(pev[m][:], lo, bt2[:, 0:512],
                                             start=False, stop=False)
                            nc.tensor.matmul(pod[m][:], lo, bt2[:, 512:1024],
                                             start=False, stop=False)
                            nc.tensor.matmul(ptl[m][:], lo, bt2[:, 1024:1376],
                                             start=False, stop=False)

                # ---- drain + interleave even/odd, DMA out. The main banks
                # (outputs 0:1024) store while the tail banks still drain ----
                for m in range(2):
                    dma = nc.sync.dma_start
                    osb = res.tile([128, OPC], f32, tag=f"osb{m}", name=f"osb{m}")
                    ov = osb[:].rearrange("p (o t) -> p o t", t=2)
                    nc.scalar.copy(ov[:, 0:512, 0], pev[m][:])
                    nc.vector.tensor_copy(ov[:, 0:512, 1], pod[m][:])
                    dma(out_d[m * 128:(m + 1) * 128, 0:1024], osb[:, 0:1024])
                    nc.scalar.copy(ov[:, 512:BPC, 0], ptl[m][:, 0:176])
                    nc.vector.tensor_copy(ov[:, 512:BPC, 1], ptl[m][:, 176:352])
                    dma(out_d[m * 128:(m + 1) * 128, 1024:OPC], osb[:, 1024:OPC])

    if compile_:
        nc.compile()
    return nc


def _host_prep(x, qweight, qzeros, scales, lora_A, lora_B):
    idx = _row_perm()                                   # (32, 128)

    # x.T rows permuted -> [128, 32*256] fp16 (shared by all cores)
    xr = x[:, idx.reshape(-1)]                          # (256, 32*128)
    xr = xr.reshape(TOKENS, NCHUNK, 128).transpose(2, 1, 0)  # (128, 32, 256)
    xt_h = np.ascontiguousarray(xr.reshape(128, NCHUNK * TOKENS)).astype(np.float16)

    # [E | lora_A.T] rows permuted -> [128, 32*48] fp16 (shared)
    i_all = np.arange(IN_F)
    E = (i_all[:, None] // GROUP == np.arange(NG)[None, :]).astype(np.float32)
    AE = np.concatenate([E, lora_A.T.astype(np.float32)], axis=1)  # (4096, 48)
    aer = AE[idx.reshape(-1)].reshape(NCHUNK, 128, AUXW).transpose(1, 0, 2)
    ae_h = np.ascontiguousarray(aer.reshape(128, NCHUNK * AUXW)).astype(np.float16)

    # per-core z4 (from qzeros bytes): even = low nibble, odd = high
    qz_b = qzeros.view(np.uint8).reshape(NG, OUT_F // 2)       # (32, 5504)
    bt2_full = (2.0 * lora_B.T).astype(np.float32)             # (16, 11008)

    in_maps = []
    for core in range(NCORES):
        o0 = core * OPC
        w0 = core * WPC
        qwc = qweight[:, w0:w0 + WPC]                          # (4096, 172)
        qwr = qwc[idx.reshape(-1)].reshape(NCHUNK, 128, WPC).transpose(1, 0, 2)
        qw_h = np.ascontiguousarray(qwr.reshape(128, NCHUNK * WPC))

        sc = scales[:, o0:o0 + OPC]                            # (32, 1376) f32
        s_ev, s_od = sc[:, 0::2], sc[:, 1::2]                  # (32, 688)
        srep_h = np.concatenate(
            [np.repeat(s_ev, 4, axis=0), np.repeat(s_od, 4, axis=0)],
            axis=1).astype(np.float16)                         # (128, 1376)

        def seg4(ev, od):
            # [ev 0:512 | od 0:512 | ev 512:688 | od 512:688] — matches the
            # on-device W/psum layout.
            return np.concatenate(
                [ev[:, :512], od[:, :512], ev[:, 512:], od[:, 512:]],
                axis=1).astype(np.float16)

        zb = qz_b[:, w0 * 4:(w0 + WPC) * 4]                    # (32, 688) bytes
        z_ev = (zb & 0xF).astype(np.float32)
        z_od = (zb >> 4).astype(np.float32)
        szn_h = seg4(-(s_ev * z_ev), -(s_od * z_od))

        btc = bt2_full[:, o0:o0 + OPC]
        bt2_h = seg4(btc[:, 0::2], btc[:, 1::2])

        in_maps.append({
            "xt": xt_h, "qw": qw_h, "srep": srep_h, "ae": ae_h,
            "szn": szn_h, "bt2": bt2_h,
        })
    return in_maps


def kernel(x, qweight, qzeros, scales, lora_A, lora_B):
    x = np.asarray(x, dtype=np.float32)
    qweight = np.ascontiguousarray(np.asarray(qweight, dtype=np.int32))
    qzeros = np.ascontiguousarray(np.asarray(qzeros, dtype=np.int32))
    scales = np.asarray(scales, dtype=np.float32)
    lora_A = np.asarray(lora_A, dtype=np.float32)
    lora_B = np.asarray(lora_B, dtype=np.float32)

    in_maps = _host_prep(x, qweight, qzeros, scales, lora_A, lora_B)
    if "nc" not in _cache:
        _cache["nc"] = build_program()
    res = run_bass_kernel_spmd(_cache["nc"], in_maps, core_ids=list(range(NCORES)))
    out = np.concatenate(
        [res.results[i]["out"] for i in range(NCORES)], axis=1)
    return np.ascontiguousarray(out.astype(np.float32))

